# revision 6
# baseline (speedup 1.0000x reference)
# BitLinear 1.58 (ternary-weight linear with int8 activation quantization)
# on 8 Trainium2 NeuronCores via Bass/Tile.
#
# Reference computation (fp32):
#   w_scale = max(mean(|W|), 1e-5)           (global over the full weight)
#   W_q     = clip(round(W / w_scale), -1, 1)          (ternary)
#   gamma   = max(max(|x|), 1e-5)            (global over the full activation)
#   x_q     = clip(round(x * 128/gamma), -128, 127)
#   out     = (x_q @ W_q^T) * (gamma*w_scale/128) + bias
#
# Sharding: data-parallel over the 8192 tokens (1024 tokens/core), weight
# replicated. Global scales via one 8-byte AllGather of per-core partials
# (local absmax(x shard), local sum|W-slice| over a distinct 1/8 of W).
#
# Schedule (v2, rebuilt from the v1 trace):
#  - v1 lost ~113us of PE idle to the stats phase: 24 MiB of stats reads at
#    ~230 GB/s (4KB-run descriptors + contention with eagerly-prefetched
#    main-loop DMAs), then a ~35us AllGather with all DMA idle.
#  - Stats reads now use flat contiguous views (8-32KB runs/partition),
#    split across both HWDGE rings (sync + scalar), w-slice first so the
#    ACT abs-accum finishes early. Main-loop W DMAs share the stats tile
#    pool (WAR serialization) and the x re-read is dep-gated, so stats get
#    the full HBM BW; W/x prefetch then floods the AllGather wait window.
#  - Main loop: token-halved PSUM rotation (4 accumulating + 4 evicting
#    banks) so no column-boundary PE stalls; ternarize batched on
#    [128,2048] tiles (4 k-slices per op) to cut per-op overhead, spread
#    over ACT+DVE; x-quantize runs entirely on DVE. x_q's upper clip to
#    127 is skipped: round(x*128/gamma) == 128 only at the global argmax
#    element(s), and bf16 holds 128 exactly; the output error is <= 1
#    integer count (~1e-4 relative), far inside the 2e-2 gate.
#  - Quantized operands in bf16 (exact: x_q in [-128,128], W_q in {-1,0,1},
#    PSUM accumulates fp32; sums bounded well under 2^24).
#  - Rounding: round-half-even in fp32 via the magic constant
#    (v + 1.5*2^23) - 1.5*2^23, fused into tensor_scalar/activation ops.

import numpy as np
from contextlib import ExitStack

import concourse.bass as bass
import concourse.tile as tile
from concourse import bacc, mybir
from concourse import bass_utils

N_CORES = 8
IN_F = 4096
OUT_F = 4096
TOKENS = 8192  # 4 * 2048
TPC = TOKENS // N_CORES  # tokens per core = 1024
OSL = OUT_F // N_CORES  # per-core weight-stats slice = 512 out_features

KT = IN_F // 128  # 32 k-tiles
KG = KT // 4  # 8 k-groups of 4 (ternarize batch)
CT = OUT_F // 512  # 8 of-columns
TT = TPC // 128  # 8 token-tiles (two halves of 4)

MAGIC = 12582912.0  # 1.5 * 2**23: (v + MAGIC) - MAGIC == round-half-even(v)
EPS = 1e-5
F32 = mybir.dt.float32
BF16 = mybir.dt.bfloat16

NXC = 16  # x-stats chunks ([128, 2048] = 1 MiB each)
NWC = 8  # w-stats chunks ([128, 2048] = 1 MiB each)

_cache = {}


def _build(dbg=False):
    nc = bacc.Bacc("TRN2", target_bir_lowering=False, debug=False, num_devices=N_CORES)
    xT = nc.dram_tensor("xT", [IN_F, TPC], F32, kind="ExternalInput").ap()
    wT = nc.dram_tensor("wT", [IN_F, OUT_F], F32, kind="ExternalInput").ap()
    wS = nc.dram_tensor("wS", [IN_F, OSL], F32, kind="ExternalInput").ap()
    bias = nc.dram_tensor("bias", [OUT_F], F32, kind="ExternalInput").ap()
    out = nc.dram_tensor("out", [TPC, OUT_F], F32, kind="ExternalOutput").ap()
    if dbg:
        dbg_t = nc.dram_tensor("dbg", [16], F32, kind="ExternalOutput").ap()

    with tile.TileContext(nc) as tc, ExitStack() as ctx:
        ep = ctx.enter_context
        singles = ep(tc.tile_pool(name="singles", bufs=1))
        # stream pool: stats chunks AND win big-tiles cycle through the same
        # slots -> main W stream naturally serializes behind the stats reads
        # (WAR on pool slots), so stats get full HBM BW.
        stream_pool = ep(tc.tile_pool(name="stream", bufs=4))
        xin_pool = ep(tc.tile_pool(name="xin", bufs=8))
        xq_pool = ep(tc.tile_pool(name="xq", bufs=KT))
        wq_pool = ep(tc.tile_pool(name="wq", bufs=9))
        ost_pool = ep(tc.tile_pool(name="ost", bufs=3))
        psum_pool = ep(tc.tile_pool(name="psum", bufs=8, space="PSUM"))
        dram = ep(tc.tile_pool(name="dram", bufs=1, space="DRAM"))

        ones_row = singles.tile([1, 128], F32)  # for partition-broadcast matmul
        nc.vector.memset(ones_row[:], 1.0)

        # ---- stats reads: flat contiguous views, big runs per partition ----
        # xT flat [128, 32768]: partition p = rows 32p..32p+31 (128KB contig);
        # 16 chunks of [128, 2048] (8KB runs). wS flat [128, 16384]: 8 chunks
        # of [128, 2048]. w first so its ACT abs-accum finishes early.
        xv = xT[:].rearrange("(p x) y -> p (x y)", p=128)
        wv = wS[:].rearrange("(p x) y -> p (x y)", p=128)

        xm = singles.tile([128, NXC], F32)
        wm = singles.tile([128, NWC], F32)
        XC = IN_F * TPC // 128 // NXC  # 2048
        WC = IN_F * OSL // 128 // NWC  # 2048
        last_stats = {}
        for j in range(NWC):
            st = stream_pool.tile([128, WC], F32, tag="stream", name=f"sw{j}")
            eng = nc.sync if j % 2 == 0 else nc.scalar
            last_stats[j % 2] = eng.dma_start(st[:], wv[:, j * WC : (j + 1) * WC])
            nc.scalar.activation(
                st[:], st[:], mybir.ActivationFunctionType.Abs,
                accum_out=wm[:, j : j + 1],
            )
        for j in range(NXC):
            st = stream_pool.tile([128, XC], F32, tag="stream", name=f"sx{j}")
            eng = nc.sync if j % 2 == 0 else nc.scalar
            last_stats[j % 2] = eng.dma_start(st[:], xv[:, j * XC : (j + 1) * XC])
            nc.vector.tensor_reduce(
                xm[:, j : j + 1], st[:], axis=mybir.AxisListType.X,
                op=mybir.AluOpType.max, apply_absolute_value=True,
            )

        # fold [128,N] -> [128,1]; cross-partition via SWDGE transpose DMA
        fold2 = singles.tile([128, 2], F32)
        nc.vector.tensor_reduce(
            fold2[:, 0:1], xm[:], axis=mybir.AxisListType.X, op=mybir.AluOpType.max
        )
        nc.vector.tensor_reduce(
            fold2[:, 1:2], wm[:], axis=mybir.AxisListType.X, op=mybir.AluOpType.add
        )
        xmaxT = singles.tile([1, 128], F32)
        nc.gpsimd.dma_start(xmaxT[:], fold2[:, 0:1])
        wsumT = singles.tile([1, 128], F32)
        nc.gpsimd.dma_start(wsumT[:], fold2[:, 1:2])
        cc_sb = singles.tile([1, 2], F32)
        nc.vector.tensor_reduce(
            cc_sb[0:1, 0:1], xmaxT[:], axis=mybir.AxisListType.X,
            op=mybir.AluOpType.max,
        )
        nc.vector.tensor_reduce(
            cc_sb[0:1, 1:2], wsumT[:], axis=mybir.AxisListType.X,
            op=mybir.AluOpType.add,
        )

        # ---- share both partial stats: one 8-byte-per-core AllGather ----
        cc_in = dram.tile([2], F32)
        cc_out = dram.tile([2 * N_CORES], F32)
        nc.gpsimd.dma_start(cc_in[:], cc_sb[:])
        nc.gpsimd.collective_compute(
            "AllGather", mybir.AluOpType.bypass,
            replica_groups=[list(range(N_CORES))],
            ins=[cc_in.opt()], outs=[cc_out.opt()],
        )
        g16 = singles.tile([1, 2 * N_CORES], F32)
        nc.gpsimd.dma_start(g16[:], cc_out[:])
        g3 = g16[:].rearrange("p (r two) -> p two r", two=2)

        # ---- bias replicated across partitions (via K=1 matmul broadcast) ----
        # staged in halves through the recycled stream pool to save SBUF
        bias_rep = singles.tile([128, OUT_F], F32)
        for h in range(2):
            bsb = stream_pool.tile([1, OUT_F // 2], F32, tag="stream", name=f"bsb{h}")
            nc.gpsimd.dma_start(bsb[:], bias[h * (OUT_F // 2) : (h + 1) * (OUT_F // 2)])
            for n in range(CT // 2):
                of = h * (OUT_F // 2) + n * 512
                bp = psum_pool.tile([128, 512], F32, tag="ps", name=f"biasps{h}_{n}")
                nc.tensor.matmul(
                    bp[:], ones_row[:], bsb[0:1, n * 512 : (n + 1) * 512],
                    start=True, stop=True,
                )
                nc.scalar.copy(bias_rep[:, of : of + 512], bp[:])

        # ---- combine gathered stats; per-partition scalar math ----
        gsum = singles.tile([1, 1], F32)
        nc.vector.tensor_reduce(
            gsum[:], g3[0:1, 1:2, :], axis=mybir.AxisListType.X,
            op=mybir.AluOpType.add,
        )
        wscale = singles.tile([1, 1], F32)
        nc.vector.tensor_scalar(
            wscale[:], gsum[:], 1.0 / (OUT_F * IN_F), EPS,
            mybir.AluOpType.mult, mybir.AluOpType.max,
        )
        gmax = singles.tile([1, 1], F32)
        nc.vector.tensor_reduce(
            gmax[:], g3[0:1, 0:1, :], axis=mybir.AxisListType.X,
            op=mybir.AluOpType.max,
        )
        gamma = singles.tile([1, 1], F32)
        nc.vector.tensor_scalar(gamma[:], gmax[:], EPS, None, mybir.AluOpType.max)

        def newton_recip(name, src):
            # correctly-rounded-ish 1/src: HW reciprocal + one Newton step
            r0 = singles.tile([1, 1], F32, tag=f"{name}r0")
            nc.vector.reciprocal(r0[:], src[:])
            t = singles.tile([1, 1], F32, tag=f"{name}t")
            nc.vector.tensor_tensor(t[:], src[:], r0[:], op=mybir.AluOpType.mult)
            u = singles.tile([1, 1], F32, tag=f"{name}u")
            nc.vector.tensor_scalar(
                u[:], t[:], -1.0, 2.0, mybir.AluOpType.mult, mybir.AluOpType.add
            )
            r1 = singles.tile([1, 1], F32, tag=f"{name}r1")
            nc.vector.tensor_tensor(r1[:], r0[:], u[:], op=mybir.AluOpType.mult)
            return r1

        rw = newton_recip("rw", wscale)  # 1/w_scale
        rg = newton_recip("rg", gamma)   # 1/gamma
        pack3 = singles.tile([1, 3], F32)
        nc.vector.tensor_scalar(
            pack3[0:1, 0:1], rg[:], 128.0, None, mybir.AluOpType.mult
        )
        nc.vector.tensor_copy(pack3[0:1, 1:2], rw[:])
        gws = singles.tile([1, 1], F32)
        nc.vector.tensor_tensor(gws[:], gamma[:], wscale[:], op=mybir.AluOpType.mult)
        nc.vector.tensor_scalar(
            pack3[0:1, 2:3], gws[:], 2.0 ** -7, None, mybir.AluOpType.mult
        )
        # broadcast [s_x, r_w, s_o] to all partitions via a K=1 PE matmul
        bp3 = psum_pool.tile([128, 3], F32, tag="ps", name="bp3")
        nc.tensor.matmul(bp3[:], ones_row[:], pack3[:], start=True, stop=True)
        b3 = singles.tile([128, 3], F32)
        nc.vector.tensor_copy(b3[:], bp3[:])
        s_x = b3[:, 0:1]
        r_w = b3[:, 1:2]
        s_o = b3[:, 2:3]

        if dbg:
            dsb = singles.tile([1, 16], F32)
            nc.vector.memset(dsb[:], 0.0)
            nc.vector.tensor_copy(dsb[0:1, 0:1], gamma[:])
            nc.vector.tensor_copy(dsb[0:1, 1:2], wscale[:])
            nc.vector.tensor_copy(dsb[0:1, 2:5], b3[96:97, :])
            nc.sync.dma_start(dbg_t[:], dsb[:])

        # ---- main loop ----
        xq = [None] * KT

        def emit_xq(k):
            # x requantize read; both rings, first ones gated behind stats
            xin = xin_pool.tile([128, TPC], F32, tag="xin", name=f"xin_q{k}")
            eng = nc.sync if k % 2 == 0 else nc.scalar
            xin_dma = eng.dma_start(xin[:], xT[k * 128 : (k + 1) * 128, :])
            if k < 8:
                for ring in (0, 1):
                    tile.add_dep_helper(
                        xin_dma.ins, last_stats[ring].ins, sync=True,
                        reason="hold x re-read until stats reads finish",
                    )
            # 2 fused DVE ops: t = x*s_x + MAGIC (rounds to int, half-even);
            # xq = t - MAGIC (no upper clip needed, see header)
            nc.vector.tensor_scalar(
                xin[:], xin[:], s_x, MAGIC, mybir.AluOpType.mult,
                mybir.AluOpType.add,
            )
            xq_k = xq_pool.tile([128, TPC], BF16, tag="xq", name=f"xq{k}")
            nc.vector.tensor_scalar(
                xq_k[:], xin[:], MAGIC, None, mybir.AluOpType.subtract,
            )
            xq[k] = xq_k

        def emit_wq(c, g):
            # one DMA brings 4 k-slices [128, 2048]; batched 3-op ternarize:
            # ACT: t = w*r_w + MAGIC; DVE: clip to MAGIC+-1; ACT/DVE: -MAGIC
            win = stream_pool.tile(
                [128, 2048], F32, tag="stream", name=f"win_c{c}_g{g}"
            )
            src = wT[g * 512 : (g + 1) * 512, c * 512 : (c + 1) * 512]
            eng = nc.sync if g % 2 == 0 else nc.scalar
            eng.dma_start(
                win[:].rearrange("p (x y) -> p x y", y=512),
                src.rearrange("(x p) y -> p x y", p=128),
            )
            nc.scalar.activation(
                win[:], win[:], mybir.ActivationFunctionType.Copy, scale=r_w,
                bias=MAGIC,
            )
            nc.vector.tensor_scalar(
                win[:], win[:], MAGIC + 1.0, MAGIC - 1.0, mybir.AluOpType.min,
                mybir.AluOpType.max,
            )
            wq = wq_pool.tile([128, 2048], BF16, tag="wq", name=f"wq_c{c}_g{g}")
            if g % 2 == 0:
                nc.vector.tensor_scalar(
                    wq[:], win[:], MAGIC, None, mybir.AluOpType.subtract
                )
            else:
                nc.scalar.activation(
                    wq[:], win[:], mybir.ActivationFunctionType.Copy, bias=-MAGIC
                )
            return wq

        def emit_evict(c, t, psum_t):
            of = c * 512
            osb = ost_pool.tile([128, 512], F32, tag="ost", name=f"osb_c{c}_t{t}")
            # out = psum * s_o + bias, one DVE op straight from PSUM
            nc.vector.scalar_tensor_tensor(
                osb[:], psum_t[:], s_o, bias_rep[:, of : of + 512],
                op0=mybir.AluOpType.mult, op1=mybir.AluOpType.add,
            )
            nc.scalar.dma_start(
                out[t * 128 : (t + 1) * 128, of : of + 512], osb[:]
            )

        prev = None  # (c, half_t0, psums) awaiting evict
        for c in range(CT):
            wqs = [None] * KG
            for half in range(2):
                t0 = half * 4
                psums = [
                    psum_pool.tile(
                        [128, 512], F32, tag="ps", name=f"ps_c{c}_t{t0 + i}"
                    )
                    for i in range(4)
                ]
                for k in range(KT):
                    if c == 0 and half == 0:
                        emit_xq(k)
                    if half == 0 and k % 4 == 0:
                        wqs[k // 4] = emit_wq(c, k // 4)
                    # previous half's evicts, one per 8 k-steps: banks free
                    # gradually without a DVE burst
                    if prev is not None and k % 8 == 4:
                        pc, pt0, pp = prev
                        i = (k - 4) // 8
                        emit_evict(pc, pt0 + i, pp[i])
                    wq_s = wqs[k // 4][:, (k % 4) * 512 : (k % 4 + 1) * 512]
                    for i in range(4):
                        t = t0 + i
                        nc.tensor.matmul(
                            psums[i][:], xq[k][:, t * 128 : (t + 1) * 128], wq_s,
                            start=(k == 0), stop=(k == KT - 1),
                        )
                prev = (c, t0, psums)
        pc, pt0, pp = prev
        for i in range(4):
            emit_evict(pc, pt0 + i, pp[i])

    nc.compile()
    return nc


def _prep_inputs(x, weight, bias):
    x2 = np.ascontiguousarray(x.reshape(TOKENS, IN_F).T)  # [IN_F, TOKENS]
    wT = np.ascontiguousarray(weight.T)  # [IN_F, OUT_F]
    in_maps = []
    for i in range(N_CORES):
        in_maps.append(
            {
                "xT": np.ascontiguousarray(x2[:, i * TPC : (i + 1) * TPC]),
                "wT": wT,
                "wS": np.ascontiguousarray(wT[:, i * OSL : (i + 1) * OSL]),
                "bias": bias,
            }
        )
    return in_maps


def _run(x, weight, bias, trace=False):
    if "nc" not in _cache:
        _cache["nc"] = _build()
    nc = _cache["nc"]
    in_maps = _prep_inputs(
        np.asarray(x, dtype=np.float32),
        np.asarray(weight, dtype=np.float32),
        np.asarray(bias, dtype=np.float32),
    )
    res = bass_utils.run_bass_kernel_spmd(
        nc, in_maps, list(range(N_CORES)), trace=trace
    )
    full = np.concatenate(
        [res.results[i]["out"] for i in range(N_CORES)], axis=0
    )
    return full.reshape(4, 2048, OUT_F), res


def kernel(x, weight, bias):
    out, _ = _run(x, weight, bias)
    return out


# revision 7
# speedup vs baseline: 1.1298x; 1.1298x over previous
# BitLinear 1.58 (ternary-weight linear with int8 activation quantization)
# on 8 Trainium2 NeuronCores via Bass/Tile.
#
# Reference computation (fp32):
#   w_scale = max(mean(|W|), 1e-5)           (global over the full weight)
#   W_q     = clip(round(W / w_scale), -1, 1)          (ternary)
#   gamma   = max(max(|x|), 1e-5)            (global over the full activation)
#   x_q     = clip(round(x * 128/gamma), -128, 127)
#   out     = (x_q @ W_q^T) * (gamma*w_scale/128) + bias
#
# Sharding: data-parallel over the 8192 tokens (1024 tokens/core), weight
# replicated. Global scales via one 8-byte AllGather of per-core partials
# (local absmax(x shard), local sum|W-slice| over a distinct 1/8 of W).
#
# Schedule (v3, rebuilt from v1/v2 traces):
#  - bias_rep is built FIRST (staged through the first stream-pool slots)
#    so the PE queue retires those matmuls immediately and the runtime
#    barrier preceding the collective clears early on every core (in v2 it
#    sat behind stats-gated bias matmuls for 122us).
#  - Stats reads: x as 16 flat [128,2048] chunks (8KB runs), w-slice with
#    the v1 chunking (16x [128,1024], preserved exactly so the fp32
#    partial-sum order -- and hence w_scale's last ulp -- matches the
#    passing v1 run; a 1-ulp w_scale shift flips boundary weights and
#    costs ~100x in max-err). Both HWDGE rings carry half of each.
#  - Main-loop DMAs are kept off the HBM during stats: win tiles are
#    dep-gated on the last stats DMAs, xin re-reads gated likewise; both
#    then flood the AllGather wait window.
#  - Ternarize: batched [128,2048] tiles (4 k-slices), all three passes on
#    DVE (~3.2us/tile vs PE consumption 3.46us/tile; v2 had pass1 on ACT
#    at ~3.5us/tile which starved the PE into HAM cold oscillation, 371us
#    throttled). x-quantize: ACT pass1 + DVE pass2 (exact v1 numerics).
#  - Token-halved PSUM rotation (4 accumulating + 4 evicting banks), one
#    evict per 8 k-steps, so no column-boundary PE bursts.
#
# Quantized operands in bf16 (exact: x_q in [-128,127], W_q in {-1,0,1},
# PSUM accumulates fp32, sums bounded by 4096*128 = 2^19 < 2^24).
# Rounding: round-half-even in fp32 via the magic constant
# (v + 1.5*2^23) - 1.5*2^23, fused into tensor_scalar/activation ops.

import numpy as np
from contextlib import ExitStack

import concourse.bass as bass
import concourse.tile as tile
from concourse import bacc, mybir
from concourse import bass_utils

N_CORES = 8
IN_F = 4096
OUT_F = 4096
TOKENS = 8192  # 4 * 2048
TPC = TOKENS // N_CORES  # tokens per core = 1024
OSL = OUT_F // N_CORES  # per-core weight-stats slice = 512 out_features

KT = IN_F // 128  # 32 k-tiles
KG = KT // 4  # 8 k-groups of 4 (ternarize batch)
CT = OUT_F // 512  # 8 of-columns
TT = TPC // 128  # 8 token-tiles (two halves of 4)

MAGIC = 12582912.0  # 1.5 * 2**23: (v + MAGIC) - MAGIC == round-half-even(v)
EPS = 1e-5
F32 = mybir.dt.float32
BF16 = mybir.dt.bfloat16

NXC = 16  # x-stats chunks [128, 2048]
NWC = 16  # w-stats chunks [128, 1024] (v1 chunking, keeps w_scale bit-exact)

_cache = {}


def _build(dbg=False):
    nc = bacc.Bacc("TRN2", target_bir_lowering=False, debug=False, num_devices=N_CORES)
    xT = nc.dram_tensor("xT", [IN_F, TPC], F32, kind="ExternalInput").ap()
    wT = nc.dram_tensor("wT", [IN_F, OUT_F], F32, kind="ExternalInput").ap()
    wS = nc.dram_tensor("wS", [IN_F, OSL], F32, kind="ExternalInput").ap()
    bias = nc.dram_tensor("bias", [OUT_F], F32, kind="ExternalInput").ap()
    out = nc.dram_tensor("out", [TPC, OUT_F], F32, kind="ExternalOutput").ap()
    if dbg:
        dbg_t = nc.dram_tensor("dbg", [16], F32, kind="ExternalOutput").ap()

    with tile.TileContext(nc) as tc, ExitStack() as ctx:
        ep = ctx.enter_context
        singles = ep(tc.tile_pool(name="singles", bufs=1))
        # stream pool: bias staging + stats chunks cycle through these slots
        stream_pool = ep(tc.tile_pool(name="stream", bufs=4))
        win_pool = ep(tc.tile_pool(name="win", bufs=3))
        xin_pool = ep(tc.tile_pool(name="xin", bufs=6))
        xq_pool = ep(tc.tile_pool(name="xq", bufs=KT))
        wq_pool = ep(tc.tile_pool(name="wq", bufs=9))
        ost_pool = ep(tc.tile_pool(name="ost", bufs=3))
        psum_pool = ep(tc.tile_pool(name="psum", bufs=8, space="PSUM"))
        dram = ep(tc.tile_pool(name="dram", bufs=1, space="DRAM"))

        ones_row = singles.tile([1, 128], F32)  # for partition-broadcast matmul
        nc.vector.memset(ones_row[:], 1.0)

        # ---- bias replicated across partitions, FIRST so the PE queue
        # drains immediately and the pre-collective barrier clears early ----
        bias_rep = singles.tile([128, OUT_F], F32)
        for h in range(2):
            bsb = stream_pool.tile(
                [1, OUT_F // 2], F32, tag="stream", name=f"bsb{h}"
            )
            nc.gpsimd.dma_start(
                bsb[:], bias[h * (OUT_F // 2) : (h + 1) * (OUT_F // 2)]
            )
            for n in range(CT // 2):
                of = h * (OUT_F // 2) + n * 512
                bp = psum_pool.tile([128, 512], F32, tag="ps", name=f"biasps{h}_{n}")
                nc.tensor.matmul(
                    bp[:], ones_row[:], bsb[0:1, n * 512 : (n + 1) * 512],
                    start=True, stop=True,
                )
                nc.scalar.copy(bias_rep[:, of : of + 512], bp[:])

        # ---- stats reads ----
        # x: flat [128, 32768] view (partition p = rows 32p..32p+31, 128KB
        # contiguous), 16 chunks of [128,2048] (8KB runs). order-independent
        # (max). w: v1 chunking, 16x [128,1024] via "(a p x) y" view -- sum
        # order preserved so w_scale matches v1 bit-for-bit.
        xv = xT[:].rearrange("(p x) y -> p (x y)", p=128)
        wv = wS[:].rearrange("(a p x) y -> a p (x y)", p=128, x=2)

        xm = singles.tile([128, NXC], F32)
        wm = singles.tile([128, NWC], F32)
        XC = IN_F * TPC // 128 // NXC  # 2048
        last_stats = {}
        for j in range(NWC):
            st = stream_pool.tile([128, 1024], F32, tag="stream", name=f"sw{j}")
            eng = nc.sync if j % 2 == 0 else nc.scalar
            last_stats[j % 2] = eng.dma_start(st[:], wv[j])
            nc.scalar.activation(
                st[:], st[:], mybir.ActivationFunctionType.Abs,
                accum_out=wm[:, j : j + 1],
            )
        for j in range(NXC):
            st = stream_pool.tile([128, XC], F32, tag="stream", name=f"sx{j}")
            eng = nc.sync if j % 2 == 0 else nc.scalar
            last_stats[j % 2] = eng.dma_start(st[:], xv[:, j * XC : (j + 1) * XC])
            nc.vector.tensor_reduce(
                xm[:, j : j + 1], st[:], axis=mybir.AxisListType.X,
                op=mybir.AluOpType.max, apply_absolute_value=True,
            )

        # fold [128,N] -> [128,1] -> cross-partition via DMA reshape -> [1,1]
        xmax = singles.tile([128, 1], F32)
        nc.vector.tensor_reduce(
            xmax[:], xm[:], axis=mybir.AxisListType.X, op=mybir.AluOpType.max
        )
        wsumc = singles.tile([128, 1], F32)
        nc.vector.tensor_reduce(
            wsumc[:], wm[:], axis=mybir.AxisListType.X, op=mybir.AluOpType.add
        )
        xmaxT = singles.tile([1, 128], F32)
        nc.gpsimd.dma_start(xmaxT[:], xmax[:])
        gx = singles.tile([1, 1], F32)
        nc.vector.tensor_reduce(
            gx[:], xmaxT[:], axis=mybir.AxisListType.X, op=mybir.AluOpType.max
        )
        wsumT = singles.tile([1, 128], F32)
        nc.gpsimd.dma_start(wsumT[:], wsumc[:])
        wsum = singles.tile([1, 1], F32)
        nc.vector.tensor_reduce(
            wsum[:], wsumT[:], axis=mybir.AxisListType.X, op=mybir.AluOpType.add
        )

        # ---- share both partial stats: one 8-byte-per-core AllGather ----
        cc_sb = singles.tile([1, 2], F32)
        nc.vector.tensor_copy(cc_sb[0:1, 0:1], gx[:])
        nc.vector.tensor_copy(cc_sb[0:1, 1:2], wsum[:])
        cc_in = dram.tile([2], F32)
        cc_out = dram.tile([2 * N_CORES], F32)
        nc.gpsimd.dma_start(cc_in[:], cc_sb[:])
        nc.gpsimd.collective_compute(
            "AllGather", mybir.AluOpType.bypass,
            replica_groups=[list(range(N_CORES))],
            ins=[cc_in.opt()], outs=[cc_out.opt()],
        )
        g16 = singles.tile([1, 2 * N_CORES], F32)
        nc.gpsimd.dma_start(g16[:], cc_out[:])
        g3 = g16[:].rearrange("p (r two) -> p two r", two=2)

        # ---- combine gathered stats; per-partition scalar math ----
        gsum = singles.tile([1, 1], F32)
        nc.vector.tensor_reduce(
            gsum[:], g3[0:1, 1:2, :], axis=mybir.AxisListType.X,
            op=mybir.AluOpType.add,
        )
        wscale = singles.tile([1, 1], F32)
        nc.vector.tensor_scalar(
            wscale[:], gsum[:], 1.0 / (OUT_F * IN_F), EPS,
            mybir.AluOpType.mult, mybir.AluOpType.max,
        )
        gmax = singles.tile([1, 1], F32)
        nc.vector.tensor_reduce(
            gmax[:], g3[0:1, 0:1, :], axis=mybir.AxisListType.X,
            op=mybir.AluOpType.max,
        )
        gamma = singles.tile([1, 1], F32)
        nc.vector.tensor_scalar(gamma[:], gmax[:], EPS, None, mybir.AluOpType.max)

        def newton_recip(name, src):
            # correctly-rounded-ish 1/src: HW reciprocal + one Newton step
            r0 = singles.tile([1, 1], F32, tag=f"{name}r0")
            nc.vector.reciprocal(r0[:], src[:])
            t = singles.tile([1, 1], F32, tag=f"{name}t")
            nc.vector.tensor_tensor(t[:], src[:], r0[:], op=mybir.AluOpType.mult)
            u = singles.tile([1, 1], F32, tag=f"{name}u")
            nc.vector.tensor_scalar(
                u[:], t[:], -1.0, 2.0, mybir.AluOpType.mult, mybir.AluOpType.add
            )
            r1 = singles.tile([1, 1], F32, tag=f"{name}r1")
            nc.vector.tensor_tensor(r1[:], r0[:], u[:], op=mybir.AluOpType.mult)
            return r1

        rw = newton_recip("rw", wscale)  # 1/w_scale
        rg = newton_recip("rg", gamma)   # 1/gamma
        pack3 = singles.tile([1, 3], F32)
        nc.vector.tensor_scalar(
            pack3[0:1, 0:1], rg[:], 128.0, None, mybir.AluOpType.mult
        )
        nc.vector.tensor_copy(pack3[0:1, 1:2], rw[:])
        gws = singles.tile([1, 1], F32)
        nc.vector.tensor_tensor(gws[:], gamma[:], wscale[:], op=mybir.AluOpType.mult)
        nc.vector.tensor_scalar(
            pack3[0:1, 2:3], gws[:], 2.0 ** -7, None, mybir.AluOpType.mult
        )
        # broadcast [s_x, r_w, s_o] to all partitions via a K=1 PE matmul
        bp3 = psum_pool.tile([128, 3], F32, tag="ps", name="bp3")
        nc.tensor.matmul(bp3[:], ones_row[:], pack3[:], start=True, stop=True)
        b3 = singles.tile([128, 3], F32)
        nc.vector.tensor_copy(b3[:], bp3[:])
        s_x = b3[:, 0:1]
        r_w = b3[:, 1:2]
        s_o = b3[:, 2:3]

        if dbg:
            dsb = singles.tile([1, 16], F32)
            nc.vector.memset(dsb[:], 0.0)
            nc.vector.tensor_copy(dsb[0:1, 0:1], gamma[:])
            nc.vector.tensor_copy(dsb[0:1, 1:2], wscale[:])
            nc.vector.tensor_copy(dsb[0:1, 2:5], b3[96:97, :])
            nc.sync.dma_start(dbg_t[:], dsb[:])

        # ---- main loop ----
        xq = [None] * KT

        def emit_xq(k):
            # x requantize read; both rings, first ones gated behind stats
            xin = xin_pool.tile([128, TPC], F32, tag="xin", name=f"xin_q{k}")
            eng = nc.sync if k % 2 == 0 else nc.scalar
            xin_dma = eng.dma_start(xin[:], xT[k * 128 : (k + 1) * 128, :])
            if k < 6:
                for ring in (0, 1):
                    tile.add_dep_helper(
                        xin_dma.ins, last_stats[ring].ins, sync=True,
                        reason="hold x re-read until stats reads finish",
                    )
            # t = x*s_x + MAGIC (ACT, rounds to int); xq = min(t-M, 127) (DVE)
            nc.scalar.activation(
                xin[:], xin[:], mybir.ActivationFunctionType.Copy, scale=s_x,
                bias=MAGIC,
            )
            xq_k = xq_pool.tile([128, TPC], BF16, tag="xq", name=f"xq{k}")
            nc.vector.tensor_scalar(
                xq_k[:], xin[:], MAGIC, 127.0, mybir.AluOpType.subtract,
                mybir.AluOpType.min,
            )
            xq[k] = xq_k

        def emit_wq(c, g):
            # one DMA brings 4 k-slices [128, 2048]; 3 fused DVE passes:
            # t = w*r_w + MAGIC; clip to MAGIC+-1; -MAGIC (cast bf16)
            win = win_pool.tile([128, 2048], F32, tag="win", name=f"win_c{c}_g{g}")
            src = wT[g * 512 : (g + 1) * 512, c * 512 : (c + 1) * 512]
            eng = nc.sync if g % 2 == 0 else nc.scalar
            win_dma = eng.dma_start(
                win[:].rearrange("p (x y) -> p x y", y=512),
                src.rearrange("(x p) y -> p x y", p=128),
            )
            if c == 0 and g < 3:
                for ring in (0, 1):
                    tile.add_dep_helper(
                        win_dma.ins, last_stats[ring].ins, sync=True,
                        reason="hold weight prefetch until stats reads finish",
                    )
            nc.vector.tensor_scalar(
                win[:], win[:], r_w, MAGIC, mybir.AluOpType.mult,
                mybir.AluOpType.add,
            )
            nc.vector.tensor_scalar(
                win[:], win[:], MAGIC + 1.0, MAGIC - 1.0, mybir.AluOpType.min,
                mybir.AluOpType.max,
            )
            wq = wq_pool.tile([128, 2048], BF16, tag="wq", name=f"wq_c{c}_g{g}")
            nc.vector.tensor_scalar(
                wq[:], win[:], MAGIC, None, mybir.AluOpType.subtract
            )
            return wq

        def emit_evict(c, t, psum_t):
            of = c * 512
            osb = ost_pool.tile([128, 512], F32, tag="ost", name=f"osb_c{c}_t{t}")
            # out = psum * s_o + bias, one DVE op straight from PSUM
            nc.vector.scalar_tensor_tensor(
                osb[:], psum_t[:], s_o, bias_rep[:, of : of + 512],
                op0=mybir.AluOpType.mult, op1=mybir.AluOpType.add,
            )
            eng = nc.scalar if t % 2 == 0 else nc.sync
            eng.dma_start(out[t * 128 : (t + 1) * 128, of : of + 512], osb[:])

        prev = None  # (c, half_t0, psums) awaiting evict
        for c in range(CT):
            wqs = [None] * KG
            for half in range(2):
                t0 = half * 4
                psums = [
                    psum_pool.tile(
                        [128, 512], F32, tag="ps", name=f"ps_c{c}_t{t0 + i}"
                    )
                    for i in range(4)
                ]
                for k in range(KT):
                    if c == 0 and half == 0:
                        emit_xq(k)
                    if half == 0 and k % 4 == 0:
                        wqs[k // 4] = emit_wq(c, k // 4)
                    # previous half's evicts, one per 8 k-steps: banks free
                    # gradually without a DVE burst
                    if prev is not None and k % 8 == 4:
                        pc, pt0, pp = prev
                        i = (k - 4) // 8
                        emit_evict(pc, pt0 + i, pp[i])
                    wq_s = wqs[k // 4][:, (k % 4) * 512 : (k % 4 + 1) * 512]
                    for i in range(4):
                        t = t0 + i
                        nc.tensor.matmul(
                            psums[i][:], xq[k][:, t * 128 : (t + 1) * 128], wq_s,
                            start=(k == 0), stop=(k == KT - 1),
                        )
                prev = (c, t0, psums)
        pc, pt0, pp = prev
        for i in range(4):
            emit_evict(pc, pt0 + i, pp[i])

    nc.compile()
    return nc


def _prep_inputs(x, weight, bias):
    x2 = np.ascontiguousarray(x.reshape(TOKENS, IN_F).T)  # [IN_F, TOKENS]
    wT = np.ascontiguousarray(weight.T)  # [IN_F, OUT_F]
    in_maps = []
    for i in range(N_CORES):
        in_maps.append(
            {
                "xT": np.ascontiguousarray(x2[:, i * TPC : (i + 1) * TPC]),
                "wT": wT,
                "wS": np.ascontiguousarray(wT[:, i * OSL : (i + 1) * OSL]),
                "bias": bias,
            }
        )
    return in_maps


def _run(x, weight, bias, trace=False):
    if "nc" not in _cache:
        _cache["nc"] = _build()
    nc = _cache["nc"]
    in_maps = _prep_inputs(
        np.asarray(x, dtype=np.float32),
        np.asarray(weight, dtype=np.float32),
        np.asarray(bias, dtype=np.float32),
    )
    res = bass_utils.run_bass_kernel_spmd(
        nc, in_maps, list(range(N_CORES)), trace=trace
    )
    full = np.concatenate(
        [res.results[i]["out"] for i in range(N_CORES)], axis=0
    )
    return full.reshape(4, 2048, OUT_F), res


def kernel(x, weight, bias):
    out, _ = _run(x, weight, bias)
    return out
